# revision 6
# baseline (speedup 1.0000x reference)
"""Trainium2 Bass kernel for nn_DRA_52905407152670 — v2.

3-layer aspect-attention GRU over (B,S,H)=(64,512,768), data-parallel over
batch across 8 cores (NB=8/core). Major structure vs v1:

- th16[b] = se + c_t is the RUNNING pre-tanh tensor: built once by a fused
  PSUM-evict (pse*(1/16) + c0), updated per layer with the transposed delta
  cT_t - cT_{t-1} (A cancels for t=2). No separate se16, no big per-layer
  re-add of c.
- fp8 DoubleRow everywhere accuracy allows (sim-validated): Se (x8@16ws),
  scores (th8 @ 64w), at (block-diag mw8 @ x8s), gi (16at8 @ 16wih8).
  gh/c-chains stay f16 (whh/wd/wa/wd1/whs f16; h state f32/f16).
- at is ONE batched matmul chain over a block-diagonal (s-part) mw8 lhsT
  with x in a second fp8 s-part layout (x8s); scale 16/(ssum*den) folded
  into mwf so the at eviction is a plain copy.
- layer-0 fully software-pipelined into the Se stream; gh chains emitted at
  layer start so they run under the tanh/add window; stationary-major
  matmul emission to enable LDWEIGHTS reuse.
- host pre-arranges every weight layout (no on-device preamble transposes).
"""
import json as _json
import sys as _sys

_sys.path.insert(0, '/opt/trn_rl_repo')

from concourse import tile as _tile_mod
from concourse import mybir as _mybir
from concourse.tile import ScopedClock as _ScopedClock

_MAX_WAITS = 1
_ws_counter = [0]


def _patched_drain_and_barrier(self, tick_clock, wait_clock):
    nc = self.nc
    carrier = nc.sync.nop(nofuse=True, hint="drain_wait_carrier")
    wait_clock.add_sem_waits(carrier.ins,
                             _ScopedClock({None: tick_clock.global_clock}))
    si = carrier.ins.sync_info
    waits = list(si.on_wait) if si is not None else []
    if len(waits) > _MAX_WAITS:
        carrier.ins.sync_info = _mybir.SyncInfo(
            on_wait=waits[:_MAX_WAITS], on_update=list(si.on_update))
        rest = waits[_MAX_WAITS:]
        for i in range(0, len(rest), _MAX_WAITS):
            extra = nc.sync.nop(nofuse=True, hint=f"drain_wait_{i}")
            extra.ins.sync_info = _mybir.SyncInfo(
                on_wait=rest[i:i + _MAX_WAITS], on_update=[])
    nc.sync.drain()
    nc.all_engine_barrier()
    assert self.sems is not None
    popped = nc._tile_sem_poison_stack.pop()
    assert popped is self._sem_poison
    nc.clear_and_free_semaphores(list(self.sems.allocated().values()))
    nc.all_engine_barrier()


_tile_mod.TileContext._drain_and_barrier = _patched_drain_and_barrier


def _split_bir_waits(bir_str):
    d = _json.loads(bir_str)
    changed = False
    for fn in d.get('functions', []):
        for blk in fn.get('blocks', []):
            out = []
            for inst in blk.get('instructions', []):
                si = inst.get('sync_info') or {}
                waits = si.get('on_wait') or []
                if len(waits) > _MAX_WAITS:
                    changed = True
                    excess, keep = waits[:-_MAX_WAITS], waits[-_MAX_WAITS:]
                    for i in range(0, len(excess), _MAX_WAITS):
                        _ws_counter[0] += 1
                        out.append({
                            "debug": inst.get("debug", 0),
                            "engine": inst["engine"],
                            "ins": [], "outs": [],
                            "name": f"I-wsplit{_ws_counter[0]}",
                            "opcode": "NoOp",
                            "sync_info": {"on_update": [],
                                          "on_wait": excess[i:i + _MAX_WAITS]},
                            "text_hint": "wait_split",
                        })
                    si = dict(si)
                    si['on_wait'] = keep
                    inst = dict(inst)
                    inst['sync_info'] = si
                out.append(inst)
            blk['instructions'] = out
    return _json.dumps(d) if changed else bir_str


import concourse.bass2jax as _b2j
import concourse.bass_utils as _bu

_orig_compile = _bu.compile_bir_kernel


def _patched_compile(bir_str, *a, **k):
    was_bytes = isinstance(bir_str, (bytes, bytearray))
    out = _split_bir_waits(bir_str.decode() if was_bytes else bir_str)
    return _orig_compile(out.encode() if was_bytes else out, *a, **k)


if getattr(_bu.compile_bir_kernel, '__name__', '') != '_patched_compile':
    _bu.compile_bir_kernel = _patched_compile
    _b2j.compile_bir_kernel = _patched_compile




import numpy as np
import concourse.bass as bass
import concourse.mybir as mybir
from concourse import tile
from concourse.masks import make_identity

dt = mybir.dt
AF = mybir.ActivationFunctionType
ALU = mybir.AluOpType
AX = mybir.AxisListType
DR = mybir.MatmulPerfMode.DoubleRow
P = 128

DEBUG = False


def chunks(total, maxc=512):
    out, c0 = [], 0
    while c0 < total:
        cl = min(maxc, total - c0)
        out.append((c0, cl))
        c0 += cl
    return out


def build_nc(NB, S, H, G, LAYERS, NCORES=8):
    KS, SB, J2, U2 = H // P, S // P, H // (2 * P), S // (2 * P)
    G3 = 3 * G
    f8, f16, f32 = dt.float8e4, dt.float16, dt.float32
    nc = bass.Bass("TRN2", target_bir_lowering=False, debug=False,
                   num_devices=NCORES)

    ap = {}
    ap['xt8'] = nc.declare_dram_parameter("xt8", [NB, P, KS, S], f8, isOutput=False)
    ap['x8s'] = nc.declare_dram_parameter("x8s", [P, NB, U2, 2, H], f8, isOutput=False)
    ap['ws8'] = nc.declare_dram_parameter("ws8", [P, KS, H], f8, isOutput=False)
    ap['wS8'] = nc.declare_dram_parameter("wS8", [P, NB, J2, 2, 16], f8, isOutput=False)
    ap['srT16'] = nc.declare_dram_parameter("srT16", [P, KS, NB], f16, isOutput=False)
    ap['aspT16'] = nc.declare_dram_parameter("aspT16", [P, KS, NB], f16, isOutput=False)
    ap['wa16'] = nc.declare_dram_parameter("wa16", [P, KS, H], f16, isOutput=False)
    ap['wd116'] = nc.declare_dram_parameter("wd116", [P, KS, H], f16, isOutput=False)
    ap['wd16'] = nc.declare_dram_parameter("wd16", [P, KS, H], f16, isOutput=False)
    ap['whs16'] = nc.declare_dram_parameter("whs16", [P, KS, G], f16, isOutput=False)
    ap['wih8'] = nc.declare_dram_parameter("wih8", [P, KS, G3], f8, isOutput=False)
    ap['whh16'] = nc.declare_dram_parameter("whh16", [P, KS, G3], f16, isOutput=False)
    ap['mask8'] = nc.declare_dram_parameter("mask8", [NB, S], f16, isOutput=False)
    ap['out'] = nc.declare_dram_parameter("out", [NB, G], f16, isOutput=True)
    if DEBUG:
        for nm, shp, dtp in (
                ('dbg_th16_0', [P, KS * S], f16), ('dbg_th8_0', [P, KS * S], f16),
                ('dbg_scps0', [NB, S], f32), ('dbg_m0', [NB, S], f16),
                ('dbg_mwf0', [NB, S], f16), ('dbg_at', [NB, H], f16),
                ('dbg_cT0', [P, KS, NB], f32), ('dbg_h0', [NB, G], f32),
                ('dbg_A', [NB, H], f16), ('dbg_hL0', [NB, G], f16),
                ('dbg_hL1', [NB, G], f16), ('dbg_rz0', [NB, 2 * G], f16),
                ('dbg_dT1', [P, KS, NB], f32)):
            ap[nm] = nc.declare_dram_parameter(nm, shp, dtp, isOutput=True)

    with tile.TileContext(nc) as tc:
        _emit(tc, nc, ap, NB, S, H, G, LAYERS)
    return nc


def _emit(tc, nc, ap, NB, S, H, G, LAYERS):
    KS, SB, J2, U2 = H // P, S // P, H // (2 * P), S // (2 * P)
    G3 = 3 * G
    f8, f16, f32 = dt.float8e4, dt.float16, dt.float32
    from contextlib import ExitStack
    ctx = ExitStack()

    # ---------------- resident tiles ----------------
    res = ctx.enter_context(tc.tile_pool(name="res", bufs=1))
    ident16 = res.tile([P, P], f16, tag="id16", name="ident16")
    make_identity(nc, ident16)

    th16 = [res.tile([P, KS * S], f16, tag=f"th16_{b}", name=f"th16_{b}")
            for b in range(NB)]
    x8s = res.tile([P, NB, U2, 2, H], f8, tag="x8s", name="x8s")
    guard = res.tile([1, 16], f16, tag="guard", name="guard")
    wih = res.tile([P, KS, G3], f8, tag="wih", name="wih")
    whh = res.tile([P, KS, G3], f16, tag="whh", name="whh")
    wd = res.tile([P, KS, H], f16, tag="wd", name="wd")
    wS = res.tile([P, NB, J2, 2, 16], f8, tag="wS", name="wS")
    srT = res.tile([P, KS, NB], f16, tag="srT", name="srT")
    aspT = res.tile([P, KS, NB], f16, tag="aspT", name="aspT")
    mask8t = res.tile([NB, S], f16, tag="mask8", name="mask8t")
    dens8 = res.tile([NB, 1], f32, tag="dens8", name="dens8")
    A16 = res.tile([NB, H], f16, tag="A16", name="A16")
    c0_16 = res.tile([NB, H], f16, tag="c0_16", name="c0_16")
    cT0 = res.tile([P, KS, NB], f32, tag="cT0", name="cT0")
    dT = res.tile([P, KS, NB], f32, tag="dT", name="dT")
    Dcum = res.tile([P, KS, NB], f32, tag="Dcum", name="Dcum")
    h16 = [res.tile([NB, G], f16, tag=f"h16_{i}", name=f"h16_{i}")
           for i in range(2)]  # ping-pong f16 state (unscaled)
    hT16 = res.tile([P, KS, NB], f16, tag="hT16", name="hT16")
    atL = res.tile([P, NB, U2, 2, 16], f8, tag="atL", name="atL")
    atTd = res.tile([P, J2, 2, 16], f8, tag="atTd", name="atTd")
    asb16 = res.tile([NB, H], f16, tag="asb16", name="asb16")

    nc.vector.memset(atL[:, :, :, :, :], 0.0)
    nc.vector.memset(atTd[:, :, :, :], 0.0)

    # ---------------- DMA issues ----------------

    # phase-A pools (explicitly closed after layer 0)
    phA_mgr = tc.tile_pool(name="phA", bufs=1)
    phA = phA_mgr.__enter__()
    ws8 = phA.tile([P, KS, H], f8, tag="ws8", name="ws8")
    wa = phA.tile([P, KS, H], f16, tag="wa", name="wa")
    wd1 = phA.tile([P, KS, H], f16, tag="wd1", name="wd1")
    whs = phA.tile([P, KS, G], f16, tag="whs", name="whs")
    xt8 = [phA.tile([P, KS, S], f8, tag=f"xt8_{b % 4}", name=f"xt8_{b}")
           for b in range(NB)]

    # ONE gpsimd queue in strict need-order: FIFO within a queue is the only
    # reliable DMA prioritization. Bulk late-use weights are dependency-staged
    # behind the first eviction via a guard op further down.
    nc.gpsimd.dma_start(out=srT[:, :, :], in_=ap['srT16'][:, :, :])
    nc.gpsimd.dma_start(out=aspT[:, :, :], in_=ap['aspT16'][:, :, :])
    nc.gpsimd.dma_start(out=wa[:, :, :], in_=ap['wa16'][:, :, :])
    nc.gpsimd.dma_start(out=ws8[:, :, :], in_=ap['ws8'][:, :, :])
    nc.gpsimd.dma_start(out=xt8[0][:, :, :], in_=ap['xt8'][0])
    nc.gpsimd.dma_start(out=wd1[:, :, :], in_=ap['wd116'][:, :, :])
    nc.gpsimd.dma_start(out=xt8[1][:, :, :], in_=ap['xt8'][1])
    nc.gpsimd.dma_start(out=wS[:, :, :, :, :], in_=ap['wS8'][:, :, :, :, :])
    nc.gpsimd.dma_start(out=mask8t[:, :], in_=ap['mask8'][:, :])
    for b in (2, 3):
        nc.gpsimd.dma_start(out=xt8[b][:, :, :], in_=ap['xt8'][b])
    nc.gpsimd.dma_start(out=whs[:, :, :], in_=ap['whs16'][:, :, :])
    for b in (4, 5, 6, 7):
        nc.gpsimd.dma_start(out=xt8[b][:, :, :], in_=ap['xt8'][b])

    # denominators (scaled 1/16 so recip = 16/(ssum*den))
    nc.vector.tensor_reduce(out=dens8, in_=mask8t[:, :], axis=AX.X, op=ALU.add)
    nc.vector.tensor_scalar_mul(dens8[:, :], dens8[:, :], 1.0 / 16)

    # ---------------- psum pools ----------------
    # bank budget (8 x 2KB): phase A: pse 3 + chain 2 + tp 2 + sc 1 = 8
    psA_mgr = tc.tile_pool(name="psA", bufs=1, space="PSUM")
    psA = psA_mgr.__enter__()
    pse = [psA.tile([P, S], f32, tag=f"pse{k}", name=f"pse{k}") for k in range(3)]
    chain = psA.tile([NB, H], f32, tag="chain", name="chain")
    scA = psA.tile([16, S], f32, tag="scA", name="scA")

    def tp_tile(pool):
        # shared (128,128) f16 transpose scratch; small transposes use [:, 0:NB]
        return pool.tile([P, P], f16, tag="tp", bufs=2,
                         name=f"tp_{nc.next_id()}")

    # ---- helpers ----
    def emit_chain_f16(ps, lhsT, rhs, width, stop_end=True):
        # lhsT: [P, KS, NB] tile; rhs: [P, KS, width]; out ps[0:NB, :width]
        # memset-based accumulation: start always False, stop on last block.
        for hs in range(KS):
            for (c0, cl) in chunks(width):
                nc.tensor.matmul(ps[0:NB, c0:c0 + cl],
                                 lhsT=lhsT[:, hs, :],
                                 rhs=rhs[:, hs, c0:c0 + cl],
                                 start=False,
                                 stop=(stop_end and hs == KS - 1),
                                 skip_group_check=True)

    def emit_se_b_ks(b, ks):
        t = pse[ks % 3]
        for j2 in range(J2):
            nc.tensor.matmul(t[:, :],
                             lhsT=ws8[:, 2 * j2:2 * j2 + 2, ks * P:(ks + 1) * P],
                             rhs=xt8[b][:, 2 * j2:2 * j2 + 2, :],
                             start=(j2 == 0), stop=(j2 == J2 - 1),
                             perf_mode=DR, skip_group_check=True)

    def emit_evict(b, ks):
        nc.vector.tensor_scalar(th16[b][:, ks * S:(ks + 1) * S],
                                pse[ks % 3][:, :], 1.0 / 16,
                                cT0[:, ks, b:b + 1],
                                op0=ALU.mult, op1=ALU.add)

    th8p = [None] * NB

    def emit_tanh(pool, b):
        t8 = pool.tile([P, KS * S], f8, tag=f"th8_{b % 3}", name=f"th8L_{b}")
        th8p[b] = t8
        nc.scalar.activation(t8[:, :], th16[b][:, :], AF.Tanh)

    def emit_tanh_biased(pool, b):
        # th16 stays frozen at se+c0; layer delta added on DVE into a scratch
        # tile (out != in keeps the 2x DVE mode), then one big ACT tanh.
        t8 = pool.tile([P, KS * S], f8, tag=f"th8_{b % 3}", name=f"th8B_{b}")
        th8p[b] = t8
        sc16 = pool.tile([P, KS * S], f16, tag=f"thsc_{b % 2}",
                         name=f"thsc_{b}")
        for ks in range(KS):
            nc.vector.tensor_scalar_add(sc16[:, ks * S:(ks + 1) * S],
                                        th16[b][:, ks * S:(ks + 1) * S],
                                        Dcum[:, ks, b:b + 1])
        nc.scalar.activation(t8[:, :], sc16[:, :], AF.Tanh)

    def emit_scores_b(ps, b, start, stop):
        t8v = th8p[b][:, :].rearrange("p (a j s) -> p a j s", a=J2, j=2)
        for j2 in range(J2):
            nc.tensor.matmul(ps[0:16, :],
                             lhsT=wS[:, b, j2, :, :],
                             rhs=t8v[:, j2, :, :],
                             start=(start and j2 == 0),
                             stop=(stop and j2 == J2 - 1),
                             perf_mode=DR, skip_group_check=True)

    def emit_hT(cur, pool):
        # hT16 = 16*h (scale folded into the psum->sbuf copy)
        for hs in range(KS):
            tph = tp_tile(pool)
            nc.tensor.transpose(tph[:, 0:NB], h16[cur][:, hs * P:(hs + 1) * P],
                                ident16[0:NB, 0:NB])
            nc.vector.tensor_scalar_mul(hT16[:, hs, :], tph[:, 0:NB], 16.0)

    # ---- phase A pipeline, batch-outer ----
    # A-chain first (its inputs land before ws8/xt0), then Se-b0, then c0.
    nc.vector.memset(chain[:, :], 0.0)
    emit_chain_f16(chain, aspT, wa, H, stop_end=False)
    nc.vector.tensor_copy(A16[:, :], chain[0:NB, :])
    if DEBUG:
        nc.sync.dma_start(out=ap['dbg_A'][:, :], in_=A16[:, :])
    for ks in range(4):
        emit_se_b_ks(0, ks)
    emit_chain_f16(chain, srT, wd1, H)
    nc.vector.tensor_copy(c0_16[:, :], chain[0:NB, :])
    for hs in range(KS):
        tpc = tp_tile(psA)
        nc.tensor.transpose(tpc[:, 0:NB], c0_16[:, hs * P:(hs + 1) * P],
                            ident16[0:NB, 0:NB])
        nc.vector.tensor_copy(cT0[:, hs, :], tpc[:, 0:NB])
    if DEBUG:
        nc.sync.dma_start(out=ap['dbg_cT0'][:, :, :], in_=cT0[:, :, :])

    for ks in range(4):
        emit_evict(0, ks)
    # staged bulk DMAs: guard depends on the first eviction, so these 11MB
    # only hit the wire once Se is streaming
    nc.gpsimd.tensor_copy(guard[0:1, :], th16[0][0:1, 0:16])
    nc.gpsimd.dma_start(out=whh[:, :, :], in_=ap['whh16'][:, :, :])
    nc.gpsimd.dma_start(out=x8s[:, :, :, :, :], in_=ap['x8s'][:, :, :, :, :])
    nc.gpsimd.dma_start(out=wih[:, :, :], in_=ap['wih8'][:, :, :])
    nc.gpsimd.dma_start(out=wd[:, :, :], in_=ap['wd16'][:, :, :])

    for ks in (4, 5):
        emit_se_b_ks(0, ks)
        emit_evict(0, ks)
    emit_tanh(phA, 0)

    for ks in range(KS):
        emit_se_b_ks(1, ks)
        emit_evict(1, ks)
    emit_tanh(phA, 1)

    # h0 chain (whs arrives mid-stream)
    nc.vector.memset(chain[:, :], 0.0)
    emit_chain_f16(chain, srT, whs, G)
    nc.vector.tensor_copy(h16[0][:, :], chain[0:NB, :])
    if DEBUG:
        nc.sync.dma_start(out=ap['dbg_h0'][:, :], in_=chain[0:NB, :])
    emit_hT(0, psA)

    for b in range(2, NB):
        for ks in range(KS):
            emit_se_b_ks(b, ks)
            emit_evict(b, ks)
        emit_tanh(phA, b)
        emit_scores_b(scA, b - 2, start=(b == 2), stop=False)

    # phase-A psum closes after the L0 softmax (scA read); psL preallocates
    # nothing until its first tile, so open it lazily below.
    psLb = {}

    def open_psL():
        # layer psum: psG 2 + sc 1 + nb768 2 + tp 2 = 7 banks
        psLb['psL'] = ctx.enter_context(
            tc.tile_pool(name="psL", bufs=1, space="PSUM"))
        psLb['psG1'] = psLb['psL'].tile([P, 512], f32, tag="psG1", name="psG1")
        psLb['psG2'] = psLb['psL'].tile([P, 512], f32, tag="psG2", name="psG2")

    def emit_gh(layer):
        psL, psG1, psG2 = psLb['psL'], psLb['psG1'], psLb['psG2']
        # gh chains (f16, stationary-major) emitted at layer start so they
        # stream under the add/tanh window. start=True on the first write of
        # each band region replaces the psG memsets (unwritten garbage rows
        # are never read).
        for hs in range(KS):
            first, last = hs == 0, hs == KS - 1
            for j in range(3):
                nc.tensor.matmul(psG1[32 * j:32 * j + NB, 0:512],
                                 lhsT=hT16[:, hs, :],
                                 rhs=whh[:, hs, 512 * j:512 * (j + 1)],
                                 start=first, stop=False,
                                 tile_position=(0, 32 * j),
                                 skip_group_check=True)
            nc.tensor.matmul(psG2[32:32 + NB, 0:512],
                             lhsT=hT16[:, hs, :],
                             rhs=whh[:, hs, 1536:2048],
                             start=first, stop=last,
                             tile_position=(0, 32),
                             skip_group_check=True)
            nc.tensor.matmul(psG2[64:64 + NB, 0:256],
                             lhsT=hT16[:, hs, :],
                             rhs=whh[:, hs, 2048:2304],
                             start=first, stop=last,
                             tile_position=(0, 64),
                             skip_group_check=True)

    # ---- scores psum for layers lives in psL ----
    scores_ps = [scA]

    def emit_scores_prep(pool):
        scores_ps[0] = psLb['psL'].tile([16, S], f32, tag="sc", bufs=1,
                                name=f"sc_{nc.next_id()}")

    def emit_softmax_exp(pool, t, ps=None, b0=0, nb=None):
        # exp + fused scale; mwf = (m*recip)*mask. Engines need aligned
        # partition bases, so halves work on rows [0:b0+nb] with only rows
        # b0.. meaningful (zero stationary cols make the rest harmless).
        ps = scores_ps[0] if ps is None else ps
        nb = NB if nb is None else nb
        rows = b0 + nb
        m16 = pool.tile([NB, S], f16, tag=f"m16_{b0}", bufs=2,
                        name=f"m16_{nc.next_id()}")
        ssum = pool.tile([NB, 1], f32, tag=f"ssum_{b0}", bufs=2,
                         name=f"ssum_{nc.next_id()}")
        nc.scalar.activation(m16[0:rows, :], ps[0:rows, :], AF.Exp,
                             scale=1.0 / 64, accum_out=ssum[0:rows, :])
        if DEBUG and t == 0 and b0 == 0 and nb == NB:
            nc.sync.dma_start(out=ap['dbg_scps0'][:, :], in_=ps[0:NB, :])
            nc.sync.dma_start(out=ap['dbg_m0'][:, :], in_=m16[:, :])
        prod = pool.tile([NB, 1], f32, tag=f"prod_{b0}", bufs=2,
                         name=f"prod_{nc.next_id()}")
        nc.vector.tensor_tensor(out=prod[0:rows, :], in0=ssum[0:rows, :],
                                in1=dens8[0:rows, :], op=ALU.mult)
        scl = pool.tile([NB, 1], f32, tag=f"scl_{b0}", bufs=2,
                        name=f"scl_{nc.next_id()}")
        nc.vector.reciprocal(out=scl[0:rows, :], in_=prod[0:rows, :])
        mwf = pool.tile([NB, S], f16, tag=f"mwf_{b0}", bufs=2,
                        name=f"mwf_{nc.next_id()}")
        nc.vector.scalar_tensor_tensor(out=mwf[0:rows, :], in0=m16[0:rows, :],
                                       scalar=scl[0:rows, 0:1],
                                       in1=mask8t[0:rows, :],
                                       op0=ALU.mult, op1=ALU.mult)
        if DEBUG and t == 0 and b0 == 0 and nb == NB:
            nc.sync.dma_start(out=ap['dbg_mwf0'][:, :], in_=mwf[:, :])
        return mwf

    def emit_mw_transposes(mwf, b0=0, nb=None):
        # transpose each s-block; scatter batch cols b0:b0+nb into atL
        nb = NB if nb is None else nb
        rows = b0 + nb
        psL = psLb['psL']
        atLf = atL[:, :, :, :, :].rearrange("p a u j c -> p (a u j c)")
        for sb in range(SB):
            u, jj = sb // 2, sb % 2
            tps = tp_tile(psL)
            nc.tensor.transpose(tps[:, 0:rows], mwf[0:rows, sb * P:(sb + 1) * P],
                                ident16[0:rows, 0:rows])
            base = 32 * u + 16 * jj + 65 * b0
            nc.vector.tensor_copy(atLf[:, base:base + 65 * (nb - 1) + 1:65],
                                  tps[:, b0:b0 + nb])

    def emit_softmax_mw(pool, t):
        emit_mw_transposes(emit_softmax_exp(pool, t))

    atps_h = [None]

    def emit_at_half(bp0):
        psL = psLb['psL']
        if bp0 == 0:
            atps_h[0] = psL.tile([16, H], f32, tag="nb768", bufs=1,
                                 name=f"atps_{nc.next_id()}")
            nc.vector.memset(atps_h[0][:, :], 0.0)
        atps = atps_h[0]
        for bp in range(bp0, bp0 + 4):
            for u in range(U2):
                for (c0, cl) in chunks(H):
                    nc.tensor.matmul(atps[0:16, c0:c0 + cl],
                                     lhsT=atL[:, bp, u, :, :],
                                     rhs=x8s[:, bp, u, :, c0:c0 + cl],
                                     start=False,
                                     stop=(bp == NB - 1 and u == U2 - 1),
                                     perf_mode=DR, skip_group_check=True)

    def emit_atTd():
        psL = psLb['psL']
        for hs in range(KS):
            tpa = tp_tile(psL)
            nc.tensor.transpose(tpa[:, 0:NB], asb16[:, hs * P:(hs + 1) * P],
                                ident16[0:NB, 0:NB])
            nc.vector.tensor_copy(atTd[:, hs // 2, hs % 2, 0:NB], tpa[:, 0:NB])

    def emit_at(pool, t):
        psL = psLb['psL']
        emit_at_half(0)
        emit_at_half(4)
        atps = atps_h[0]
        nc.vector.tensor_copy(asb16[:, :], atps[0:NB, :])
        if DEBUG and t == 0:
            nc.sync.dma_start(out=ap['dbg_at'][:, :], in_=asb16[:, :])
        emit_atTd()

    def emit_gru(pool, t):
        psL, psG1, psG2 = psLb['psL'], psLb['psG1'], psLb['psG2']
        psG3 = psL.tile([16, 512], f32, tag="psG3", bufs=1,
                        name=f"psG3_{nc.next_id()}")
        atTdf = atTd[:, :, :, :].rearrange("p a j c -> p (a j) c")
        wihd = wih[:, :, :].rearrange("p k c -> p k c")
        # gi chains (fp8, non-DR: DR can't use tile_position column offsets),
        # band-major with r/z gate math interleaved per finished band
        rz = pool.tile([NB, 2 * G], f16, tag="rz", bufs=1, name=f"rz{t}")
        for j, c0 in enumerate((0, 512, 1024)):
            for hs in range(KS):
                nc.tensor.matmul(psG1[32 * j:32 * j + 16, 0:512],
                                 lhsT=atTdf[:, hs, :],
                                 rhs=wihd[:, hs, 512 * j:512 * (j + 1)],
                                 start=False, stop=(hs == KS - 1),
                                 tile_position=(0, 32 * j),
                                 skip_group_check=True)
            # (bands already initialized by gh's start=True)
            trz = pool.tile([NB, 512], f16, tag="trz", bufs=2,
                            name=f"trz{t}_{c0}")
            nc.scalar.activation(trz[:, :], psG1[32 * j:32 * j + NB, :],
                                 AF.Tanh, scale=0.5 / 256)
            nc.vector.tensor_scalar(rz[:, c0:c0 + 512], trz[:, :],
                                    0.5, 0.5, op0=ALU.mult, op1=ALU.add)
        wihp = wih[:, :, :].rearrange("p (a j) c -> p a j c", a=J2)
        for j2 in range(J2):
            first, last = j2 == 0, j2 == J2 - 1
            nc.tensor.matmul(psG3[0:16, 0:512],
                             lhsT=atTd[:, j2, :, :],
                             rhs=wihp[:, j2, :, 1536:2048],
                             start=first, stop=last,
                             perf_mode=DR, skip_group_check=True)
            nc.tensor.matmul(psG2[0:16, 0:256],
                             lhsT=atTd[:, j2, :, :],
                             rhs=wihp[:, j2, :, 2048:2304],
                             start=first, stop=last,
                             perf_mode=DR, skip_group_check=True)
        if DEBUG and t == 0:
            nc.sync.dma_start(out=ap['dbg_rz0'][:, :], in_=rz[:, :])
        n_sb = pool.tile([NB, G], f16, tag="n_sb", bufs=1, name=f"n{t}")
        for c0, cl, psgi, psgh in (
                (0, 512, psG3[0:NB, 0:512], psG2[32:32 + NB, 0:512]),
                (512, 256, psG2[0:NB, 0:256], psG2[64:64 + NB, 0:256])):
            tmp = pool.tile([NB, 512], f32, tag="gtmp", bufs=2,
                            name=f"gtmp{t}_{c0}")
            nc.vector.tensor_tensor(out=tmp[:, :cl], in0=rz[:, c0:c0 + cl],
                                    in1=psgh, op=ALU.mult)
            nc.vector.tensor_tensor(out=tmp[:, :cl], in0=tmp[:, :cl],
                                    in1=psgi, op=ALU.add)
            nc.scalar.activation(n_sb[:, c0:c0 + cl], tmp[:, :cl],
                                 AF.Tanh, scale=1.0 / 256)
        hmn = pool.tile([NB, G], f16, tag="hmn", bufs=1, name=f"hmn{t}")
        nc.vector.tensor_tensor(out=hmn[:, :], in0=h16[t % 2][:, :],
                                in1=n_sb[:, :], op=ALU.subtract)
        nc.vector.tensor_tensor(out=hmn[:, :], in0=rz[:, G:2 * G],
                                in1=hmn[:, :], op=ALU.mult)
        nc.vector.tensor_tensor(out=h16[(t + 1) % 2][:, :], in0=n_sb[:, :],
                                in1=hmn[:, :], op=ALU.add)

    # ================= LAYER 0 tail =================
    mwf0 = emit_softmax_exp(phA, 0)
    psA_mgr.__exit__(None, None, None)
    open_psL()
    emit_gh(0)
    emit_mw_transposes(mwf0)
    emit_at(phA, 0)
    emit_gru(phA, 0)
    if DEBUG:
        nc.sync.dma_start(out=ap['dbg_hL0'][:, :], in_=h16[1][:, :])
        nc.sync.dma_start(out=ap['dbg_th16_0'][:, :], in_=th16[0][:, :])

    # ================= LAYERS 1,2 =================
    phA_mgr.__exit__(None, None, None)
    lay = ctx.enter_context(tc.tile_pool(name="lay", bufs=1))

    for t in range(1, LAYERS):
        psL = psLb['psL']
        cur, prev = t % 2, (t + 1) % 2
        emit_hT(cur, psL)

        # ---- delta chain ----
        cdps = psL.tile([NB, H], f32, tag="nb768", bufs=1, name=f"cdps{t}")
        d16 = lay.tile([NB, H], f16, tag="d16", bufs=1, name=f"d16_{t}")
        if t == 1:
            # c1 = (hT16 @ wd)/16 + A ; d = c1 - c0
            nc.vector.memset(cdps[:, :], 0.0)
            emit_chain_f16(cdps, hT16, wd, H, False)
            c1_16 = lay.tile([NB, H], f16, tag="c1_16", bufs=1, name="c1_16")
            nc.vector.scalar_tensor_tensor(out=c1_16[:, :], in0=cdps[0:NB, :],
                                           scalar=1.0 / 16, in1=A16[:, :],
                                           op0=ALU.mult, op1=ALU.add)
            nc.vector.tensor_tensor(out=d16[:, :], in0=c1_16[:, :],
                                    in1=c0_16[:, :], op=ALU.subtract)
        else:
            # d = (h_cur - h_prev) @ wd   (state unscaled f16)
            hd16 = lay.tile([NB, G], f16, tag="hd16", bufs=1, name="hd16")
            nc.vector.tensor_tensor(out=hd16[:, :], in0=h16[cur][:, :],
                                    in1=h16[prev][:, :], op=ALU.subtract)
            hdT = lay.tile([P, KS, NB], f16, tag="hdT", bufs=1, name="hdT")
            for hs in range(KS):
                tpd = tp_tile(psL)
                nc.tensor.transpose(tpd[:, 0:NB], hd16[:, hs * P:(hs + 1) * P],
                                    ident16[0:NB, 0:NB])
                nc.vector.tensor_copy(hdT[:, hs, :], tpd[:, 0:NB])
            nc.vector.memset(cdps[:, :], 0.0)
            emit_chain_f16(cdps, hdT, wd, H, False)
            nc.vector.tensor_copy(d16[:, :], cdps[0:NB, :])
        for hs in range(KS):
            tpc = tp_tile(psL)
            nc.tensor.transpose(tpc[:, 0:NB], d16[:, hs * P:(hs + 1) * P],
                                ident16[0:NB, 0:NB])
            nc.vector.tensor_copy(dT[:, hs, :], tpc[:, 0:NB])
        if t == 1:
            nc.vector.tensor_copy(Dcum[:, :, :], dT[:, :, :])
        else:
            nc.vector.tensor_tensor(out=Dcum[:, :, :], in0=Dcum[:, :, :],
                                    in1=dT[:, :, :], op=ALU.add)
        if DEBUG and t == 1:
            nc.sync.dma_start(out=ap['dbg_dT1'][:, :, :], in_=dT[:, :, :])

        emit_gh(t)

        # ---- tanh stream with interleaved A-half softmax so exp-A sits
        #      between tanh b3 and b4 on the ACT queue ----
        psL = psLb['psL']
        scA1 = psL.tile([16, S], f32, tag="sc", bufs=1,
                        name=f"scA1_{nc.next_id()}")
        # B-half chain shares psG3's bank (time-disjoint with gi_n use)
        scB1 = psL.tile([16, 512], f32, tag="psG3", bufs=1,
                        name=f"scB1_{nc.next_id()}")
        for b in range(4):
            emit_tanh_biased(lay, b)
        for b in range(4):
            emit_scores_b(scA1, b, start=(b == 0), stop=(b == 3))
        mwfA = emit_softmax_exp(lay, t, ps=scA1, b0=0, nb=4)
        for b in (4, 5):
            emit_tanh_biased(lay, b)
        for b in (4, 5):
            emit_scores_b(scB1, b, start=(b == 4), stop=False)
        emit_mw_transposes(mwfA, 0, 4)
        emit_at_half(0)
        for b in (6, 7):
            emit_tanh_biased(lay, b)
        for b in (6, 7):
            emit_scores_b(scB1, b, start=False, stop=(b == NB - 1))
        mwfB = emit_softmax_exp(lay, t, ps=scB1, b0=4, nb=4)
        emit_mw_transposes(mwfB, 4, 4)
        emit_at_half(4)
        nc.vector.tensor_copy(asb16[:, :], atps_h[0][0:NB, :])
        emit_atTd()
        emit_gru(lay, t)
        if DEBUG and t == 1:
            nc.sync.dma_start(out=ap['dbg_hL1'][:, :], in_=h16[0][:, :])

    nc.sync.dma_start(out=ap['out'][:, :], in_=h16[LAYERS % 2][:, :])
    ctx.close()


# ---------------- host side ----------------

def make_in_maps(inputs, NB, S, H, G, NCORES=8):
    f8np = mybir.dt.np(mybir.dt.float8e4)
    KS, J2, U2 = H // P, H // (2 * P), S // (2 * P)
    G3 = 3 * G
    x32 = np.asarray(inputs['sentence_embeddings'], np.float32)
    B = x32.shape[0]
    # xt8[b, p, hb, s] = x[b, s, hb*128+p]
    xt8 = np.ascontiguousarray(
        x32.transpose(0, 2, 1).reshape(B, KS, P, S).transpose(0, 2, 1, 3)
    ).astype(f8np)
    # x8s[p, b, u, jj, h] = x[b, (2u+jj)*128+p, h]  (partition-major)
    x8s = np.ascontiguousarray(
        x32.reshape(B, U2, 2, P, H).transpose(3, 0, 1, 2, 4)).astype(f8np)

    def part_layout(m, scale=1.0, dtype=np.float16):
        # m: (H_in, K) -> [P, H_in//P, K]
        Hin, K = m.shape
        return np.ascontiguousarray(
            (scale * m).reshape(Hin // P, P, K).transpose(1, 0, 2)).astype(dtype)

    ws = np.asarray(inputs['ws'], np.float32)
    wa = np.asarray(inputs['wa'], np.float32)
    wd1 = np.asarray(inputs['wd1'], np.float32)
    wdm = np.asarray(inputs['wd'], np.float32)
    whs = np.asarray(inputs['whs'], np.float32)
    wihT = np.ascontiguousarray(np.asarray(inputs['w_ih'], np.float32).T)
    whhT = np.ascontiguousarray(np.asarray(inputs['w_hh'], np.float32).T)
    w = np.asarray(inputs['w'], np.float32)
    sr = np.asarray(inputs['sentence_representation'], np.float32)
    asp = np.asarray(inputs['aspect_embedding'], np.float32)
    mask = np.asarray(inputs['attention_mask'], np.float32)

    NBH = 8
    wS8 = np.zeros((P, NBH, J2, 2, 16), np.float32)
    wr = (64.0 * w).reshape(J2, 2, P).transpose(2, 0, 1)  # [p, j2, jj]
    for b in range(NBH):
        wS8[:, b, :, :, b] = wr
    wS8 = np.ascontiguousarray(wS8).astype(f8np)
    wih8 = part_layout(wihT, 16.0, f8np)

    common = {
        'ws8': part_layout(ws, 16.0, f8np),
        'wa16': part_layout(wa),
        'wd116': part_layout(wd1),
        'wd16': part_layout(wdm),
        'whs16': part_layout(whs),
        'whh16': part_layout(whhT, 16.0),
        'wih8': wih8,
        'wS8': wS8,
    }
    in_maps = []
    for c in range(NCORES):
        sl = slice(c * NB, (c + 1) * NB)
        m = dict(common)
        m['xt8'] = np.ascontiguousarray(xt8[sl])
        m['x8s'] = np.ascontiguousarray(x8s[:, sl])
        # srT16[p, hb, b] = sr[b, hb*128+p]
        m['srT16'] = np.ascontiguousarray(
            sr[sl].reshape(NB, KS, P).transpose(2, 1, 0)).astype(np.float16)
        m['aspT16'] = np.ascontiguousarray(
            asp[sl].reshape(NB, KS, P).transpose(2, 1, 0)).astype(np.float16)
        m['mask8'] = np.ascontiguousarray(mask[sl]).astype(np.float16)
        in_maps.append(m)
    return in_maps


# --------------------------------------------------------------------------
# Harness entry point
# --------------------------------------------------------------------------
B, S_, H_, G_ = 64, 512, 768, 768
NCORES = 8
NB_ = B // NCORES

TRACE = False
TRACE_DIR = None
LAST_EXEC_NS = None

_CACHE = {}


def kernel(**inputs):
    """Full inputs in (as in setup_inputs()), full (64, 768) fp32 output."""
    global LAST_EXEC_NS
    from concourse.bass_utils import run_bass_kernel_spmd
    if 'nc' not in _CACHE:
        _CACHE['nc'] = build_nc(NB_, S_, H_, G_, 3, NCORES)
    in_maps = make_in_maps(inputs, NB_, S_, H_, G_, NCORES)
    kw = {}
    if TRACE:
        kw = dict(trace=True, tmpdir=TRACE_DIR)
    res = run_bass_kernel_spmd(_CACHE['nc'], in_maps, list(range(NCORES)), **kw)
    LAST_EXEC_NS = res.exec_time_ns
    _CACHE['results'] = res.results
    return np.concatenate([res.results[c]['out'] for c in range(NCORES)],
                          axis=0).astype(np.float32)
